# revision 29
# baseline (speedup 1.0000x reference)
"""Cross-attention kernel for 8 TRN2 NeuronCores (Bass/Tile, SPMD) — v5.

Problem: B=4, SQ=SKV=2048, D_MODEL=1024, H=16 heads, Dh=64, fp32.
    Q = q @ Wq.T + bq; K = kv @ Wk.T + bk; V = kv @ Wv.T + bv
    out = softmax(Q K^T / sqrt(Dh)) V  -> concat heads -> @ Wo.T + bo

Sharding: 8 cores = 4 batches x 2 head-groups (8 heads each); host sums the
two out-projection partials per batch (no device collectives).

v5 (from HW NTFF profile analysis of v4, which showed PE 83% busy with
3.3us stalls at every block boundary + constant HAM 4/8 re-throttling):
  - pvh PSUM accumulators double-buffered across alternate blocks (4 banks)
    so PV of block i+1 never waits for block i's reciprocal->norm flush.
  - norm flush deferred to sb==6 of the next block: the 2x3.3us DVE
    reciprocals complete before the broadcast matmul enters the PE queue
    (no more head-of-line blocking).
  - scores PSUM: 3 rotating [128,512] banks, exp reads 512-col halves;
    fillers/outproj/rb use a separate 1-bank pool (no pool contention
    with the scores pipeline).  3+1+4 = 8 PSUM banks exactly.
  - LDWEIGHTS dedup: one explicit ldweights per scores/PV pair, matmuls
    marked ldweights=False (HW profile showed a serial ~95ns LDW per MM).
  - per-iteration order: PV(i-1) then scores(i+1) then filler, which
    hides the exp WAR on the rotating scores banks.
"""

import numpy as np

B = 4
S = 2048          # SQ == SKV
D = 1024
H_PER_CORE = 8
DH = 64
DC = H_PER_CORE * DH            # 512 head-concat dims per core
DHP = DH + 1                    # V-hat column block per head (64 + ones col)
N_CORES = 8

_CACHE = {}


def _build_program(repeat=1, ablate=None, unroll=False):
    import concourse.bass as bass
    import concourse.tile as tile
    from concourse import bacc, mybir

    f32 = mybir.dt.float32
    f32r = mybir.dt.float32r
    bf16 = mybir.dt.float16  # fp16: same PE rate, 10-bit mantissa
    nc = bacc.Bacc("TRN2", target_bir_lowering=False, debug=False,
                   enable_asserts=False, num_devices=N_CORES)

    qT = nc.dram_tensor("qT", [D, S], bf16, kind="ExternalInput").ap()
    kvT = nc.dram_tensor("kvT", [D, S], bf16, kind="ExternalInput").ap()
    wqT = nc.dram_tensor("wqT", [D, DC], bf16, kind="ExternalInput").ap()
    wkT = nc.dram_tensor("wkT", [D, DC], bf16, kind="ExternalInput").ap()
    wvh = nc.dram_tensor("wvh", [D, DC], bf16, kind="ExternalInput").ap()
    bq = nc.dram_tensor("bq", [DC], f32, kind="ExternalInput").ap()
    bk = nc.dram_tensor("bk", [DC], f32, kind="ExternalInput").ap()
    bvh = nc.dram_tensor("bvh", [DC], bf16, kind="ExternalInput").ap()
    woT = nc.dram_tensor("woT", [DC, D], bf16, kind="ExternalInput").ap()
    bo = nc.dram_tensor("bo", [D], bf16, kind="ExternalInput").ap()
    out = nc.dram_tensor("out", [S, D], bf16, kind="ExternalOutput").ap()

    VW = (H_PER_CORE - 1) * DHP + 128   # 583: compact V + 128-col stationary reads (FWL)
    KC = D // 128               # 8 contraction chunks for projections
    NM = DC // 128              # 4 partition chunks of QT/KT
    SQW = 512                   # s-quarter width
    JW = 1024                   # attention q-block width
    NSB = S // 128              # 16 kv blocks
    NPAR = 1 if repeat == 1 else 2
    EXPF = mybir.ActivationFunctionType.Exp
    IDF = mybir.ActivationFunctionType.Identity

    def noload_pair(ldw_ap, mms, tile_position=(0, 0)):
        """LDW-dedup disabled: measured SLOWER on HW (619us vs 556us) —
        a standalone LDWEIGHTS serializes; self-loading matmuls overlap
        better through the PE reorder window."""

    with tile.TileContext(nc) as tc:
        with tc.tile_pool(name="persist", bufs=1) as persist, \
             tc.tile_pool(name="xx", bufs=2) as xx, \
             tc.tile_pool(name="sps", bufs=3, space="PSUM") as sps, \
             tc.tile_pool(name="pjs", bufs=1, space="PSUM") as pjs, \
             tc.tile_pool(name="pvs", bufs=1, space="PSUM") as pvs, \
             tc.tile_pool(name="ptp", bufs=2) as ptp, \
             tc.tile_pool(name="nrm", bufs=1) as nrm, \
             tc.tile_pool(name="rbp", bufs=1) as rbp, \
             tc.tile_pool(name="otp", bufs=2) as otp:

            qt_t = [[persist.tile([128, S], bf16, tag=f"qt{p}_{m}", name=f"qt{p}_{m}")
                     for m in range(NM)] for p in range(NPAR)]
            kt_t = [[persist.tile([128, S], bf16, tag=f"kt{p}_{m}", name=f"kt{p}_{m}")
                     for m in range(NM)] for p in range(NPAR)]
            vh_t = [[persist.tile([128, VW], bf16, tag=f"vh{p}_{sb}", name=f"vh{p}_{sb}")
                     for sb in range(NSB)] for p in range(NPAR)]
            at_t = [persist.tile([128, S], bf16, tag=f"at{m}", name=f"at{m}")
                    for m in range(NM)]

            bq_t = persist.tile([128, NM], f32, tag="bq")
            bk_t = persist.tile([128, NM], f32, tag="bk")
            bvh_t = persist.tile([128, DC], bf16, tag="bvh")
            bo_t = persist.tile([128, D], bf16, tag="bo")
            ones_t = persist.tile([1, DH], f32r, tag="ones")

            wq_t = [persist.tile([128, DC], bf16, tag=f"wq{k}", name=f"wq{k}") for k in range(KC)]
            wk_t = [persist.tile([128, DC], bf16, tag=f"wk{k}", name=f"wk{k}") for k in range(KC)]
            wv_t = [persist.tile([128, DC], bf16, tag=f"wv{k}", name=f"wv{k}") for k in range(KC)]
            wo_t = [persist.tile([128, D], bf16, tag=f"wo{k}", name=f"wo{k}") for k in range(NM)]

            def col_ap(vec, n):
                return bass.AP(tensor=vec.tensor, offset=vec.offset,
                               ap=[[1, 128], [128, n]])

            def bcast_ap(vec, p, w):
                return bass.AP(tensor=vec.tensor, offset=vec.offset,
                               ap=[[0, p], [1, w]])

            def weight_dmas():
                # split across both HWDGE queues (SP + ACT)
                for k in range(KC):
                    nc.sync.dma_start(out=wk_t[k], in_=wkT[k * 128:(k + 1) * 128, :])
                    nc.sync.dma_start(out=wv_t[k], in_=wvh[k * 128:(k + 1) * 128, :])
                nc.sync.dma_start(out=bk_t, in_=col_ap(bk, NM))
                nc.sync.dma_start(out=bvh_t, in_=bcast_ap(bvh, 128, DC))
                for k in range(KC):
                    nc.sync.dma_start(out=wq_t[k], in_=wqT[k * 128:(k + 1) * 128, :])
                nc.sync.dma_start(out=bq_t, in_=col_ap(bq, NM))
                for k in range(NM):
                    nc.sync.dma_start(out=wo_t[k], in_=woT[k * 128:(k + 1) * 128, :])
                nc.sync.dma_start(out=bo_t, in_=bcast_ap(bo, 128, D))
                nc.scalar.activation(ones_t, bo_t[0:1, 0:DH], IDF,
                                     bias=1.0, scale=0.0)

            def load_quarter(pool, dram, sq):
                ssl = slice(sq * SQW, (sq + 1) * SQW)
                c = [pool.tile([128, SQW], bf16, tag=f"{pool.name}{k}",
                               name=f"{pool.name}{k}")
                     for k in range(KC)]
                for k in range(KC):
                    nc.sync.dma_start(out=c[k], in_=dram[k * 128:(k + 1) * 128, ssl])
                return c

            def prep_steps(p):
                """All x loads + K/V/Q projections for parity p.  Each DVE
                bias-add is deferred to the NEXT draw so it enters the DVE
                FIFO with its matmul group already complete (a bias-add that
                waits ~3us in the FIFO head-blocks the block reciprocals
                behind it, which stalls PV two blocks later)."""
                tail = [None]

                def flush_tail():
                    if tail[0] is not None:
                        tail[0]()
                        tail[0] = None

                kv_next = load_quarter(xx, kvT, 0)
                for sq in range(S // SQW):
                    ssl = slice(sq * SQW, (sq + 1) * SQW)
                    kv_c = kv_next
                    if sq + 1 < S // SQW:
                        kv_next = load_quarter(xx, kvT, sq + 1)
                    yield
                    for m in range(NM):
                        msl = slice(m * 128, (m + 1) * 128)
                        flush_tail()
                        ps = pjs.tile([128, SQW], f32, tag="pj")
                        for k in range(KC):
                            nc.tensor.matmul(ps, wk_t[k][:, msl], kv_c[k],
                                             start=(k == 0), stop=(k == KC - 1))
                        def t(ps=ps, m=m, ssl=ssl):
                            nc.vector.tensor_scalar_add(kt_t[p][m][:, ssl], ps,
                                                        bk_t[:, m:m + 1])
                        tail[0] = t
                        yield
                    for sm in range(SQW // 128):
                        sb = sq * (SQW // 128) + sm
                        smsl = slice(sm * 128, (sm + 1) * 128)
                        flush_tail()
                        ps = pjs.tile([128, SQW], f32, tag="pj")
                        for k in range(KC):
                            nc.tensor.matmul(ps[:, 0:DC], kv_c[k][:, smsl],
                                             wv_t[k],
                                             start=(k == 0), stop=(k == KC - 1))
                        def t(ps=ps, sb=sb):
                            vh = vh_t[p][sb]
                            vh3 = bass.AP(tensor=vh.tensor, offset=vh.offset,
                                          ap=[vh.ap[0], [DHP, H_PER_CORE], [1, DH]])
                            ps3 = bass.AP(tensor=ps.tensor, offset=ps.offset,
                                          ap=[ps.ap[0], [DH, H_PER_CORE], [1, DH]])
                            bv3 = bass.AP(tensor=bvh_t.tensor, offset=bvh_t.offset,
                                          ap=[bvh_t.ap[0], [DH, H_PER_CORE], [1, DH]])
                            nc.vector.tensor_add(vh3, ps3, bv3)
                        tail[0] = t
                        yield
                q_next = load_quarter(xx, qT, 0)
                for sq in range(S // SQW):
                    ssl = slice(sq * SQW, (sq + 1) * SQW)
                    q_c = q_next
                    if sq + 1 < S // SQW:
                        q_next = load_quarter(xx, qT, sq + 1)
                    yield
                    for m in range(NM):
                        msl = slice(m * 128, (m + 1) * 128)
                        flush_tail()
                        ps = pjs.tile([128, SQW], f32, tag="pj")
                        for k in range(KC):
                            nc.tensor.matmul(ps, wq_t[k][:, msl], q_c[k],
                                             start=(k == 0), stop=(k == KC - 1))
                        def t(ps=ps, m=m, ssl=ssl):
                            nc.vector.tensor_scalar_add(qt_t[p][m][:, ssl], ps,
                                                        bq_t[:, m:m + 1])
                        tail[0] = t
                        yield
                flush_tail()

            def outproj_steps(jc):
                """Out-projection for q rows [jc*1024, jc*1024+1024).  The
                DVE bias-add (and the DMA after the second half) is deferred
                to the next draw, same rationale as prep_steps."""
                tail = [None]

                def flush_tail():
                    if tail[0] is not None:
                        tail[0]()
                        tail[0] = None

                for qm in range(jc * 8, jc * 8 + 8):
                    qsl = slice(qm * 128, (qm + 1) * 128)
                    o_t = otp.tile([128, D], bf16, tag="o")
                    for n in range(D // 512):
                        nsl = slice(n * 512, (n + 1) * 512)
                        flush_tail()
                        po = pjs.tile([128, 512], f32, tag="pj")
                        for k in range(NM):
                            nc.tensor.matmul(po, at_t[k][:, qsl], wo_t[k][:, nsl],
                                             start=(k == 0), stop=(k == NM - 1))
                        def t(po=po, o_t=o_t, nsl=nsl, qsl=qsl, last=(n == D // 512 - 1)):
                            nc.vector.tensor_add(o_t[:, nsl], po, bo_t[:, nsl])
                            if last:
                                nc.sync.dma_start(out=out[qsl, :], in_=o_t)
                        tail[0] = t
                        yield
                flush_tail()

            pending_norm = []

            def attn_half(p, fil_c, fil_d):
                """One full iteration's attention at parity p.  A block is a
                HEAD PAIR x one 512-col q-chunk: the two heads' score matmuls
                are 64-contraction row tiles at tile_position (0,0)/(64,0)
                and execute CONCURRENTLY on the PE array (LDW of one tile
                overlaps the other's matmul).  16 blocks x 16 kv steps.
                Per step i: PV(i-1), scores(i+1), one filler draw every other
                step.  pvh banks alternate by block parity; the norm flush
                for block b runs at sb==8 of block b+1 and only gates PV of
                block b+2."""
                QC = S // 512                      # 4 q-chunks
                blocks = [(hp, qc) for qc in range(QC)
                          for hp in range(H_PER_CORE // 2)]
                state = {}

                def ensure_state(bi):
                    if bi in state:
                        return
                    hp, qc = blocks[bi]
                    par = bi % 2
                    state[bi] = dict(
                        hp=hp, qc=qc,
                        pvh=[pvs.tile([128, 512], f32, tag=f"pv{par}{s}",
                                      name=f"pv{par}{s}")
                             for s in range(2)])

                def scores(bi, sb):
                    st = state[bi]
                    sbsl = slice(sb * 128, (sb + 1) * 128)
                    jsl = slice(st["qc"] * 512, (st["qc"] + 1) * 512)
                    sp = []
                    p_t = []
                    for s in range(2):
                        hsl = slice(s * 64, (s + 1) * 64)
                        spn = sps.tile([128, 512], f32, tag="sc")
                        nc.tensor.matmul(
                            spn, kt_t[p][st["hp"]][hsl, sbsl],
                            qt_t[p][st["hp"]][hsl, jsl],
                            start=True, stop=True)
                        sp.append(spn)
                    for s in range(2):
                        pt = ptp.tile([128, 512], bf16, tag=f"p{s}")
                        nc.scalar.activation(pt, sp[s], EXPF, scale=0.125)
                        p_t.append(pt)
                    return p_t

                def finish_block(bi):
                    st = state[bi]
                    recs = []
                    for s in range(2):
                        # ~51-ULP single-op reciprocal: 5x faster than
                        # InstReciprocal so it clears the DVE FIFO long
                        # before the flush.  HW quirk: the custom-DVE op
                        # misreads PSUM at base_partition 64, so stage the
                        # denominator row to SBUF first.
                        den = nrm.tile([1, 512], f32, tag="den", name="den")
                        nc.vector.tensor_scalar_add(den, st["pvh"][s][64:65, :], 0.0)
                        rec = nrm.tile([1, 512], f32, tag=f"rec{s}", name=f"rec{s}")
                        nc.vector.reciprocal_approx_fast(rec, den)
                        recs.append(rec)

                    def flush_norm():
                        jn = slice(st["qc"] * 512, (st["qc"] + 1) * 512)
                        for s in range(2):
                            # broadcast 1/denom across partitions on the idle
                            # GpSimd engine (replaces the ones-matmul + copy)
                            hsl = slice(s * 64, (s + 1) * 64)
                            rbS = rbp.tile([DH, 512], f32, tag="rbS",
                                           name="rbS")
                            nc.gpsimd.partition_broadcast(rbS, recs[s],
                                                          channels=DH)
                            nc.vector.tensor_mul(
                                at_t[st["hp"]][hsl, jn],
                                st["pvh"][s][0:DH, :], rbS)

                    pending_norm.append(flush_norm)

                def pv_of(j):
                    bj, sj = divmod(j, NSB)
                    st = state[bj]
                    p_t = p_ts.pop(j)
                    if ablate == "const_p":
                        p_t = [kt_t[0][0][:, 0:512]] * 2
                    for s in range(2):
                        h = st["hp"] * 2 + s
                        # 128-col stationary (65 real + 63 junk cols from the
                        # next head) -> FWL hides the weight load; junk lands
                        # in pvh rows 65-127 which are never read
                        vw = vh_t[p][sj][:, h * DHP:h * DHP + 128]
                        nc.tensor.matmul(
                            st["pvh"][s], vw, p_t[s],
                            start=(sj == 0), stop=(sj == NSB - 1))
                    if sj == NSB - 1:
                        finish_block(bj)
                        state.pop(bj - 2, None)

                n_iters = len(blocks) * NSB
                p_ts = {}
                ensure_state(0)
                p_ts[0] = scores(0, 0)
                for i in range(n_iters):
                    bi, sb = divmod(i, NSB)
                    if i + 1 < n_iters:
                        b1, s1 = divmod(i + 1, NSB)
                        ensure_state(b1)
                        p_ts[i + 1] = scores(b1, s1)
                    if i >= 1:
                        pv_of(i - 1)   # one-iteration lag: exp(i-1) is stale
                    if sb == 8 and pending_norm:
                        pending_norm.pop()()
                    if i % 2 == 1:
                        if fil_c is not None and next(fil_c, "end") != "end":
                            pass
                        elif i > 8 * NSB + 8 and fil_d is not None:
                            # the last q-half-0 flush is deferred to sb==8 of
                            # block 8 (i==136); outproj(0) reads at_t after it
                            next(fil_d, None)
                for _ in fil_c or ():
                    pass
                pv_of(n_iters - 1)

            import itertools

            def body(p, with_prep, prev_jc1):
                gens = []
                if prev_jc1:
                    gens.append(outproj_steps(1))
                if with_prep:
                    gens.append(prep_steps((p + 1) % NPAR))
                fil_c = itertools.chain(*gens) if gens else None
                fil_d = outproj_steps(0)
                attn_half(p, fil_c, fil_d)
                for _ in fil_d:
                    pass
                while pending_norm:
                    pending_norm.pop()()

            # prologue: weights + parity-0 projections, compact
            weight_dmas()
            for pp_ in range(NPAR):
                for sb in range(NSB):
                    # zero the whole tile once so the tail cols past the
                    # last head's 65 real cols are initialized (finite)
                    nc.scalar.activation(vh_t[pp_][sb], bo_t[:, 0:VW], IDF,
                                         bias=0.0, scale=0.0)
                    vh = vh_t[pp_][sb][:, DH:DH + 1]
                    ones_cols = bass.AP(tensor=vh.tensor, offset=vh.offset,
                                        ap=[vh.ap[0], [DHP, H_PER_CORE], [1, 1]])
                    src_ap = bass.AP(tensor=bo_t.tensor, offset=bo_t.offset,
                                     ap=[bo_t.ap[0], [1, H_PER_CORE], [1, 1]])
                    nc.scalar.activation(ones_cols, src_ap, IDF,
                                         bias=1.0, scale=0.0)
            for _ in prep_steps(0):
                pass

            if repeat == 1:
                body(0, with_prep=False, prev_jc1=False)
                for _ in outproj_steps(1):
                    pass
            elif unroll:
                assert repeat % 2 == 0, "pipelined build needs even repeat"
                for _ in range(repeat // 2):
                    body(0, with_prep=True, prev_jc1=True)
                    body(1, with_prep=True, prev_jc1=True)
                for _ in outproj_steps(1):
                    pass
                while pending_norm:
                    pending_norm.pop()()
            else:
                assert repeat % 2 == 0, "pipelined build needs even repeat"
                with tc.For_i(0, repeat // 2, 1):
                    body(0, with_prep=True, prev_jc1=True)
                    body(1, with_prep=True, prev_jc1=True)
                # the last body's jc=1 out-projection drains here
                for _ in outproj_steps(1):
                    pass
                while pending_norm:
                    pending_norm.pop()()

    nc.compile()
    return nc


def _get_runner(repeat=1, unroll=False):
    """Build the program once and return a cached jitted SPMD runner."""
    key = ("runner", repeat, unroll)
    if key in _CACHE:
        return _CACHE[key]

    import jax
    from jax.sharding import Mesh, PartitionSpec
    from jax.experimental.shard_map import shard_map
    from concourse import mybir
    from concourse.bass2jax import (_bass_exec_p, install_neuronx_cc_hook,
                                    partition_id_tensor)

    nc = _build_program(repeat, unroll=unroll)
    install_neuronx_cc_hook()

    partition_name = nc.partition_id_tensor.name if nc.partition_id_tensor else None
    in_names, out_names, out_avals, zero_shapes = [], [], [], []
    for alloc in nc.m.functions[0].allocations:
        if not isinstance(alloc, mybir.MemoryLocationSet):
            continue
        name = alloc.memorylocations[0].name
        if alloc.kind == "ExternalInput":
            if name != partition_name:
                in_names.append(name)
        elif alloc.kind == "ExternalOutput":
            out_names.append(name)
            shape = tuple(alloc.tensor_shape)
            dtype = mybir.dt.np(alloc.dtype)
            out_avals.append(jax.core.ShapedArray(shape, dtype))
            zero_shapes.append((shape, dtype))
    n_params = len(in_names)
    n_outs = len(out_avals)
    all_in_names = list(in_names) + list(out_names)
    if partition_name is not None:
        all_in_names.append(partition_name)
    donate = tuple(range(n_params, n_params + n_outs))

    def _body(*args):
        operands = list(args)
        if partition_name is not None:
            operands.append(partition_id_tensor())
        outs = _bass_exec_p.bind(
            *operands,
            out_avals=tuple(out_avals),
            in_names=tuple(all_in_names),
            out_names=tuple(out_names),
            lowering_input_output_aliases=(),
            sim_require_finite=True,
            sim_require_nnan=True,
            nc=nc,
        )
        return tuple(outs)

    devices = jax.devices()[:N_CORES]
    mesh = Mesh(np.asarray(devices), ("core",))
    in_specs = (PartitionSpec("core"),) * (n_params + n_outs)
    out_specs = (PartitionSpec("core"),) * n_outs
    sharded = jax.jit(
        shard_map(_body, mesh=mesh, in_specs=in_specs, out_specs=out_specs,
                  check_rep=False),
        donate_argnums=donate, keep_unused=True)

    def run(in_maps):
        concat_in = [np.concatenate([np.asarray(m[name]) for m in in_maps], axis=0)
                     for name in in_names]
        concat_zeros = [np.zeros((N_CORES * s[0], *s[1:]), d) for s, d in zero_shapes]
        out_arrs = sharded(*concat_in, *concat_zeros)
        out_arrs = [np.asarray(a) for a in jax.block_until_ready(out_arrs)]
        return [
            {name: out_arrs[i].reshape(N_CORES, *out_avals[i].shape)[c]
             for i, name in enumerate(out_names)}
            for c in range(N_CORES)
        ]

    _CACHE[("internals", repeat, unroll)] = {
        "sharded": sharded, "mesh": mesh, "in_names": in_names,
        "out_names": out_names, "zero_shapes": zero_shapes, "nc": nc,
    }
    _CACHE[key] = run
    return run


def _prep_in_maps(query, key_value, Wq, bq, Wk, bk, Wv, bv, Wo, bo):
    f = np.float32
    bf = np.float16
    in_maps = []
    for c in range(N_CORES):
        b, hg = c // 2, c % 2
        sl = slice(hg * DC, (hg + 1) * DC)
        wvh = np.asarray(Wv, f)[sl, :].T
        bvh = np.asarray(bv, f)[sl]
        in_maps.append({
            "qT": np.ascontiguousarray(np.asarray(query, f)[b].T).astype(bf),
            "kvT": np.ascontiguousarray(np.asarray(key_value, f)[b].T).astype(bf),
            "wqT": np.ascontiguousarray(np.asarray(Wq, f)[sl, :].T).astype(bf),
            "wkT": np.ascontiguousarray(np.asarray(Wk, f)[sl, :].T).astype(bf),
            "wvh": np.ascontiguousarray(wvh).astype(bf),
            "bq": np.ascontiguousarray(np.asarray(bq, f)[sl]),
            "bk": np.ascontiguousarray(np.asarray(bk, f)[sl]),
            "bvh": np.ascontiguousarray(bvh).astype(bf),
            "woT": np.ascontiguousarray(np.asarray(Wo, f)[:, sl].T).astype(bf),
            "bo": (np.asarray(bo, f) if hg == 0 else np.zeros(D, f)).astype(bf),
        })
    return in_maps


def kernel(query, key_value, Wq, bq, Wk, bk, Wv, bv, Wo, bo):
    run = _get_runner()
    in_maps = _prep_in_maps(query, key_value, Wq, bq, Wk, bk, Wv, bv, Wo, bo)
    results = run(in_maps)
    out = np.empty((B, S, D), np.float32)
    for b in range(B):
        out[b] = (results[2 * b]["out"].astype(np.float32)
                  + results[2 * b + 1]["out"].astype(np.float32))
    return out


# revision 31
# speedup vs baseline: 1.1329x; 1.1329x over previous
"""Cross-attention kernel for 8 TRN2 NeuronCores (Bass/Tile, SPMD) — v5.

Problem: B=4, SQ=SKV=2048, D_MODEL=1024, H=16 heads, Dh=64, fp32.
    Q = q @ Wq.T + bq; K = kv @ Wk.T + bk; V = kv @ Wv.T + bv
    out = softmax(Q K^T / sqrt(Dh)) V  -> concat heads -> @ Wo.T + bo

Sharding: 8 cores = 4 batches x 2 head-groups (8 heads each); host sums the
two out-projection partials per batch (no device collectives).

v11 = v4 + HW-NTFF-profile-driven fixes (595.7us -> 439.2us steady state,
PE 98.6% busy, HAM throttle 26%->5.5%):
  - blocks are HEAD PAIRS x 512-col q-chunk; the two heads' score matmuls
    are 64-contraction row tiles at tile_position (0,0)/(64,0).
  - pvh PSUM accumulators double-buffered across alternate blocks (4 banks)
    so PV of block b+1 never waits for block b's norm flush; flush pops at
    sb==8 of the next block.  PSUM: 3 (scores) + 1 (fillers) + 4 = 8 banks.
  - softmax denominators: reciprocal_approx_fast (0.7us vs 3.3us DVE
    InstReciprocal; staged through SBUF — the custom op misreads PSUM at
    base_partition 64), broadcast via GpSimd partition_broadcast (idle
    engine), multiply on DVE.  No PE broadcast matmul at all.
  - filler DVE tails (bias adds, outproj store) deferred one draw so they
    enter the DVE FIFO with producers complete; otherwise their ~3us FIFO
    waits head-block the block reciprocals and stall PV two blocks later.
  - per-iteration order: PV(i-1), scores(i+1), filler every other step.
  - measured dead ends: explicit ldweights + ldweights=False (slower: the
    standalone LDW serializes), fp8 DoubleRow (V quantization alone gives
    2.6e-2 > the 2e-2 gate), scores-before-PV order (487us).
"""

import numpy as np

B = 4
S = 2048          # SQ == SKV
D = 1024
H_PER_CORE = 8
DH = 64
DC = H_PER_CORE * DH            # 512 head-concat dims per core
DHP = DH + 1                    # V-hat column block per head (64 + ones col)
N_CORES = 8

_CACHE = {}


def _build_program(repeat=1, ablate=None, unroll=False):
    import concourse.bass as bass
    import concourse.tile as tile
    from concourse import bacc, mybir

    f32 = mybir.dt.float32
    f32r = mybir.dt.float32r
    bf16 = mybir.dt.float16  # fp16: same PE rate, 10-bit mantissa
    nc = bacc.Bacc("TRN2", target_bir_lowering=False, debug=False,
                   enable_asserts=False, num_devices=N_CORES)

    qT = nc.dram_tensor("qT", [D, S], bf16, kind="ExternalInput").ap()
    kvT = nc.dram_tensor("kvT", [D, S], bf16, kind="ExternalInput").ap()
    wqT = nc.dram_tensor("wqT", [D, DC], bf16, kind="ExternalInput").ap()
    wkT = nc.dram_tensor("wkT", [D, DC], bf16, kind="ExternalInput").ap()
    wvh = nc.dram_tensor("wvh", [D, DC], bf16, kind="ExternalInput").ap()
    bq = nc.dram_tensor("bq", [DC], f32, kind="ExternalInput").ap()
    bk = nc.dram_tensor("bk", [DC], f32, kind="ExternalInput").ap()
    bvh = nc.dram_tensor("bvh", [DC], bf16, kind="ExternalInput").ap()
    woT = nc.dram_tensor("woT", [DC, D], bf16, kind="ExternalInput").ap()
    bo = nc.dram_tensor("bo", [D], bf16, kind="ExternalInput").ap()
    out = nc.dram_tensor("out", [S, D], bf16, kind="ExternalOutput").ap()

    VW = (H_PER_CORE - 1) * DHP + 128   # 583: compact V + 128-col stationary reads (FWL)
    KC = D // 128               # 8 contraction chunks for projections
    NM = DC // 128              # 4 partition chunks of QT/KT
    SQW = 512                   # s-quarter width
    JW = 1024                   # attention q-block width
    NSB = S // 128              # 16 kv blocks
    NPAR = 1 if repeat == 1 else 2
    EXPF = mybir.ActivationFunctionType.Exp
    IDF = mybir.ActivationFunctionType.Identity

    def noload_pair(ldw_ap, mms, tile_position=(0, 0)):
        """LDW-dedup disabled: measured SLOWER on HW (619us vs 556us) —
        a standalone LDWEIGHTS serializes; self-loading matmuls overlap
        better through the PE reorder window."""

    with tile.TileContext(nc) as tc:
        with tc.tile_pool(name="persist", bufs=1) as persist, \
             tc.tile_pool(name="xx", bufs=2) as xx, \
             tc.tile_pool(name="sps", bufs=3, space="PSUM") as sps, \
             tc.tile_pool(name="pjs", bufs=1, space="PSUM") as pjs, \
             tc.tile_pool(name="pvs", bufs=1, space="PSUM") as pvs, \
             tc.tile_pool(name="ptp", bufs=2) as ptp, \
             tc.tile_pool(name="nrm", bufs=1) as nrm, \
             tc.tile_pool(name="rbp", bufs=1) as rbp, \
             tc.tile_pool(name="otp", bufs=2) as otp:

            qt_t = [[persist.tile([128, S], bf16, tag=f"qt{p}_{m}", name=f"qt{p}_{m}")
                     for m in range(NM)] for p in range(NPAR)]
            kt_t = [[persist.tile([128, S], bf16, tag=f"kt{p}_{m}", name=f"kt{p}_{m}")
                     for m in range(NM)] for p in range(NPAR)]
            vh_t = [[persist.tile([128, VW], bf16, tag=f"vh{p}_{sb}", name=f"vh{p}_{sb}")
                     for sb in range(NSB)] for p in range(NPAR)]
            at_t = [persist.tile([128, S], bf16, tag=f"at{m}", name=f"at{m}")
                    for m in range(NM)]

            bq_t = persist.tile([128, NM], f32, tag="bq")
            bk_t = persist.tile([128, NM], f32, tag="bk")
            bvh_t = persist.tile([128, DC], bf16, tag="bvh")
            bo_t = persist.tile([128, D], bf16, tag="bo")
            ones_t = persist.tile([1, DH], f32r, tag="ones")

            wq_t = [persist.tile([128, DC], bf16, tag=f"wq{k}", name=f"wq{k}") for k in range(KC)]
            wk_t = [persist.tile([128, DC], bf16, tag=f"wk{k}", name=f"wk{k}") for k in range(KC)]
            wv_t = [persist.tile([128, DC], bf16, tag=f"wv{k}", name=f"wv{k}") for k in range(KC)]
            wo_t = [persist.tile([128, D], bf16, tag=f"wo{k}", name=f"wo{k}") for k in range(NM)]

            def col_ap(vec, n):
                return bass.AP(tensor=vec.tensor, offset=vec.offset,
                               ap=[[1, 128], [128, n]])

            def bcast_ap(vec, p, w):
                return bass.AP(tensor=vec.tensor, offset=vec.offset,
                               ap=[[0, p], [1, w]])

            def weight_dmas():
                # split across both HWDGE queues (SP + ACT)
                for k in range(KC):
                    nc.sync.dma_start(out=wk_t[k], in_=wkT[k * 128:(k + 1) * 128, :])
                    nc.sync.dma_start(out=wv_t[k], in_=wvh[k * 128:(k + 1) * 128, :])
                nc.sync.dma_start(out=bk_t, in_=col_ap(bk, NM))
                nc.sync.dma_start(out=bvh_t, in_=bcast_ap(bvh, 128, DC))
                for k in range(KC):
                    nc.sync.dma_start(out=wq_t[k], in_=wqT[k * 128:(k + 1) * 128, :])
                nc.sync.dma_start(out=bq_t, in_=col_ap(bq, NM))
                for k in range(NM):
                    nc.sync.dma_start(out=wo_t[k], in_=woT[k * 128:(k + 1) * 128, :])
                nc.sync.dma_start(out=bo_t, in_=bcast_ap(bo, 128, D))
                nc.scalar.activation(ones_t, bo_t[0:1, 0:DH], IDF,
                                     bias=1.0, scale=0.0)

            def load_quarter(pool, dram, sq):
                ssl = slice(sq * SQW, (sq + 1) * SQW)
                c = [pool.tile([128, SQW], bf16, tag=f"{pool.name}{k}",
                               name=f"{pool.name}{k}")
                     for k in range(KC)]
                for k in range(KC):
                    nc.sync.dma_start(out=c[k], in_=dram[k * 128:(k + 1) * 128, ssl])
                return c

            def prep_steps(p):
                """All x loads + K/V/Q projections for parity p.  Each DVE
                bias-add is deferred to the NEXT draw so it enters the DVE
                FIFO with its matmul group already complete (a bias-add that
                waits ~3us in the FIFO head-blocks the block reciprocals
                behind it, which stalls PV two blocks later)."""
                tail = [None]

                def flush_tail():
                    if tail[0] is not None:
                        tail[0]()
                        tail[0] = None

                kv_next = load_quarter(xx, kvT, 0)
                for sq in range(S // SQW):
                    ssl = slice(sq * SQW, (sq + 1) * SQW)
                    kv_c = kv_next
                    if sq + 1 < S // SQW:
                        kv_next = load_quarter(xx, kvT, sq + 1)
                    yield
                    for m in range(NM):
                        msl = slice(m * 128, (m + 1) * 128)
                        flush_tail()
                        ps = pjs.tile([128, SQW], f32, tag="pj")
                        for k in range(KC):
                            nc.tensor.matmul(ps, wk_t[k][:, msl], kv_c[k],
                                             start=(k == 0), stop=(k == KC - 1))
                        def t(ps=ps, m=m, ssl=ssl):
                            nc.vector.tensor_scalar_add(kt_t[p][m][:, ssl], ps,
                                                        bk_t[:, m:m + 1])
                        tail[0] = t
                        yield
                    for sm in range(SQW // 128):
                        sb = sq * (SQW // 128) + sm
                        smsl = slice(sm * 128, (sm + 1) * 128)
                        flush_tail()
                        ps = pjs.tile([128, SQW], f32, tag="pj")
                        for k in range(KC):
                            nc.tensor.matmul(ps[:, 0:DC], kv_c[k][:, smsl],
                                             wv_t[k],
                                             start=(k == 0), stop=(k == KC - 1))
                        def t(ps=ps, sb=sb):
                            vh = vh_t[p][sb]
                            vh3 = bass.AP(tensor=vh.tensor, offset=vh.offset,
                                          ap=[vh.ap[0], [DHP, H_PER_CORE], [1, DH]])
                            ps3 = bass.AP(tensor=ps.tensor, offset=ps.offset,
                                          ap=[ps.ap[0], [DH, H_PER_CORE], [1, DH]])
                            bv3 = bass.AP(tensor=bvh_t.tensor, offset=bvh_t.offset,
                                          ap=[bvh_t.ap[0], [DH, H_PER_CORE], [1, DH]])
                            nc.vector.tensor_add(vh3, ps3, bv3)
                        tail[0] = t
                        yield
                q_next = load_quarter(xx, qT, 0)
                for sq in range(S // SQW):
                    ssl = slice(sq * SQW, (sq + 1) * SQW)
                    q_c = q_next
                    if sq + 1 < S // SQW:
                        q_next = load_quarter(xx, qT, sq + 1)
                    yield
                    for m in range(NM):
                        msl = slice(m * 128, (m + 1) * 128)
                        flush_tail()
                        ps = pjs.tile([128, SQW], f32, tag="pj")
                        for k in range(KC):
                            nc.tensor.matmul(ps, wq_t[k][:, msl], q_c[k],
                                             start=(k == 0), stop=(k == KC - 1))
                        def t(ps=ps, m=m, ssl=ssl):
                            nc.vector.tensor_scalar_add(qt_t[p][m][:, ssl], ps,
                                                        bq_t[:, m:m + 1])
                        tail[0] = t
                        yield
                flush_tail()

            def outproj_steps(jc):
                """Out-projection for q rows [jc*1024, jc*1024+1024).  The
                DVE bias-add (and the DMA after the second half) is deferred
                to the next draw, same rationale as prep_steps."""
                tail = [None]

                def flush_tail():
                    if tail[0] is not None:
                        tail[0]()
                        tail[0] = None

                for qm in range(jc * 8, jc * 8 + 8):
                    qsl = slice(qm * 128, (qm + 1) * 128)
                    o_t = otp.tile([128, D], bf16, tag="o")
                    for n in range(D // 512):
                        nsl = slice(n * 512, (n + 1) * 512)
                        flush_tail()
                        po = pjs.tile([128, 512], f32, tag="pj")
                        for k in range(NM):
                            nc.tensor.matmul(po, at_t[k][:, qsl], wo_t[k][:, nsl],
                                             start=(k == 0), stop=(k == NM - 1))
                        def t(po=po, o_t=o_t, nsl=nsl, qsl=qsl, last=(n == D // 512 - 1)):
                            nc.vector.tensor_add(o_t[:, nsl], po, bo_t[:, nsl])
                            if last:
                                nc.sync.dma_start(out=out[qsl, :], in_=o_t)
                        tail[0] = t
                        yield
                flush_tail()

            pending_norm = []

            def attn_half(p, fil_c, fil_d):
                """One full iteration's attention at parity p.  A block is a
                HEAD PAIR x one 512-col q-chunk: the two heads' score matmuls
                are 64-contraction row tiles at tile_position (0,0)/(64,0)
                and execute CONCURRENTLY on the PE array (LDW of one tile
                overlaps the other's matmul).  16 blocks x 16 kv steps.
                Per step i: PV(i-1), scores(i+1), one filler draw every other
                step.  pvh banks alternate by block parity; the norm flush
                for block b runs at sb==8 of block b+1 and only gates PV of
                block b+2."""
                QC = S // 512                      # 4 q-chunks
                blocks = [(hp, qc) for qc in range(QC)
                          for hp in range(H_PER_CORE // 2)]
                state = {}

                def ensure_state(bi):
                    if bi in state:
                        return
                    hp, qc = blocks[bi]
                    par = bi % 2
                    state[bi] = dict(
                        hp=hp, qc=qc,
                        pvh=[pvs.tile([128, 512], f32, tag=f"pv{par}{s}",
                                      name=f"pv{par}{s}")
                             for s in range(2)])

                def scores(bi, sb):
                    st = state[bi]
                    sbsl = slice(sb * 128, (sb + 1) * 128)
                    jsl = slice(st["qc"] * 512, (st["qc"] + 1) * 512)
                    sp = []
                    p_t = []
                    for s in range(2):
                        hsl = slice(s * 64, (s + 1) * 64)
                        spn = sps.tile([128, 512], f32, tag="sc")
                        nc.tensor.matmul(
                            spn, kt_t[p][st["hp"]][hsl, sbsl],
                            qt_t[p][st["hp"]][hsl, jsl],
                            start=True, stop=True)
                        sp.append(spn)
                    for s in range(2):
                        pt = ptp.tile([128, 512], bf16, tag=f"p{s}")
                        nc.scalar.activation(pt, sp[s], EXPF, scale=0.125)
                        p_t.append(pt)
                    return p_t

                def finish_block(bi):
                    st = state[bi]
                    recs = []
                    for s in range(2):
                        # ~51-ULP single-op reciprocal: 5x faster than
                        # InstReciprocal so it clears the DVE FIFO long
                        # before the flush.  HW quirk: the custom-DVE op
                        # misreads PSUM at base_partition 64, so stage the
                        # denominator row to SBUF first.
                        den = nrm.tile([1, 512], f32, tag="den", name="den")
                        nc.vector.tensor_scalar_add(den, st["pvh"][s][64:65, :], 0.0)
                        rec = nrm.tile([1, 512], f32, tag=f"rec{s}", name=f"rec{s}")
                        nc.vector.reciprocal_approx_fast(rec, den)
                        recs.append(rec)

                    def flush_norm():
                        jn = slice(st["qc"] * 512, (st["qc"] + 1) * 512)
                        for s in range(2):
                            # broadcast 1/denom across partitions on the idle
                            # GpSimd engine (replaces the ones-matmul + copy)
                            hsl = slice(s * 64, (s + 1) * 64)
                            rbS = rbp.tile([DH, 512], f32, tag="rbS",
                                           name="rbS")
                            nc.gpsimd.partition_broadcast(rbS, recs[s],
                                                          channels=DH)
                            nc.vector.tensor_mul(
                                at_t[st["hp"]][hsl, jn],
                                st["pvh"][s][0:DH, :], rbS)

                    pending_norm.append(flush_norm)

                def pv_of(j):
                    bj, sj = divmod(j, NSB)
                    st = state[bj]
                    p_t = p_ts.pop(j)
                    if ablate == "const_p":
                        p_t = [kt_t[0][0][:, 0:512]] * 2
                    for s in range(2):
                        h = st["hp"] * 2 + s
                        # 128-col stationary (65 real + 63 junk cols from the
                        # next head) -> FWL hides the weight load; junk lands
                        # in pvh rows 65-127 which are never read
                        vw = vh_t[p][sj][:, h * DHP:h * DHP + 128]
                        nc.tensor.matmul(
                            st["pvh"][s], vw, p_t[s],
                            start=(sj == 0), stop=(sj == NSB - 1))
                    if sj == NSB - 1:
                        finish_block(bj)
                        state.pop(bj - 2, None)

                n_iters = len(blocks) * NSB
                p_ts = {}
                ensure_state(0)
                p_ts[0] = scores(0, 0)
                for i in range(n_iters):
                    bi, sb = divmod(i, NSB)
                    if i >= 1:
                        pv_of(i - 1)   # one-iteration lag: exp(i-1) is stale
                    if i + 1 < n_iters:
                        b1, s1 = divmod(i + 1, NSB)
                        ensure_state(b1)
                        p_ts[i + 1] = scores(b1, s1)
                    if sb == 8 and pending_norm:
                        pending_norm.pop()()
                    if i % 2 == 1:
                        if fil_c is not None and next(fil_c, "end") != "end":
                            pass
                        elif i > 8 * NSB + 8 and fil_d is not None:
                            # the last q-half-0 flush is deferred to sb==8 of
                            # block 8 (i==136); outproj(0) reads at_t after it
                            next(fil_d, None)
                for _ in fil_c or ():
                    pass
                pv_of(n_iters - 1)

            import itertools

            def body(p, with_prep, prev_jc1):
                gens = []
                if prev_jc1:
                    gens.append(outproj_steps(1))
                if with_prep:
                    gens.append(prep_steps((p + 1) % NPAR))
                fil_c = itertools.chain(*gens) if gens else None
                fil_d = outproj_steps(0)
                attn_half(p, fil_c, fil_d)
                for _ in fil_d:
                    pass
                while pending_norm:
                    pending_norm.pop()()

            # prologue: weights + parity-0 projections, compact
            weight_dmas()
            for pp_ in range(NPAR):
                for sb in range(NSB):
                    # zero the whole tile once so the tail cols past the
                    # last head's 65 real cols are initialized (finite)
                    nc.scalar.activation(vh_t[pp_][sb], bo_t[:, 0:VW], IDF,
                                         bias=0.0, scale=0.0)
                    vh = vh_t[pp_][sb][:, DH:DH + 1]
                    ones_cols = bass.AP(tensor=vh.tensor, offset=vh.offset,
                                        ap=[vh.ap[0], [DHP, H_PER_CORE], [1, 1]])
                    src_ap = bass.AP(tensor=bo_t.tensor, offset=bo_t.offset,
                                     ap=[bo_t.ap[0], [1, H_PER_CORE], [1, 1]])
                    nc.scalar.activation(ones_cols, src_ap, IDF,
                                         bias=1.0, scale=0.0)
            for _ in prep_steps(0):
                pass

            if repeat == 1:
                body(0, with_prep=False, prev_jc1=False)
                for _ in outproj_steps(1):
                    pass
            elif unroll:
                assert repeat % 2 == 0, "pipelined build needs even repeat"
                for _ in range(repeat // 2):
                    body(0, with_prep=True, prev_jc1=True)
                    body(1, with_prep=True, prev_jc1=True)
                for _ in outproj_steps(1):
                    pass
                while pending_norm:
                    pending_norm.pop()()
            else:
                assert repeat % 2 == 0, "pipelined build needs even repeat"
                with tc.For_i(0, repeat // 2, 1):
                    body(0, with_prep=True, prev_jc1=True)
                    body(1, with_prep=True, prev_jc1=True)
                # the last body's jc=1 out-projection drains here
                for _ in outproj_steps(1):
                    pass
                while pending_norm:
                    pending_norm.pop()()

    nc.compile()
    return nc


def _get_runner(repeat=1, unroll=False):
    """Build the program once and return a cached jitted SPMD runner."""
    key = ("runner", repeat, unroll)
    if key in _CACHE:
        return _CACHE[key]

    import jax
    from jax.sharding import Mesh, PartitionSpec
    from jax.experimental.shard_map import shard_map
    from concourse import mybir
    from concourse.bass2jax import (_bass_exec_p, install_neuronx_cc_hook,
                                    partition_id_tensor)

    nc = _build_program(repeat, unroll=unroll)
    install_neuronx_cc_hook()

    partition_name = nc.partition_id_tensor.name if nc.partition_id_tensor else None
    in_names, out_names, out_avals, zero_shapes = [], [], [], []
    for alloc in nc.m.functions[0].allocations:
        if not isinstance(alloc, mybir.MemoryLocationSet):
            continue
        name = alloc.memorylocations[0].name
        if alloc.kind == "ExternalInput":
            if name != partition_name:
                in_names.append(name)
        elif alloc.kind == "ExternalOutput":
            out_names.append(name)
            shape = tuple(alloc.tensor_shape)
            dtype = mybir.dt.np(alloc.dtype)
            out_avals.append(jax.core.ShapedArray(shape, dtype))
            zero_shapes.append((shape, dtype))
    n_params = len(in_names)
    n_outs = len(out_avals)
    all_in_names = list(in_names) + list(out_names)
    if partition_name is not None:
        all_in_names.append(partition_name)
    donate = tuple(range(n_params, n_params + n_outs))

    def _body(*args):
        operands = list(args)
        if partition_name is not None:
            operands.append(partition_id_tensor())
        outs = _bass_exec_p.bind(
            *operands,
            out_avals=tuple(out_avals),
            in_names=tuple(all_in_names),
            out_names=tuple(out_names),
            lowering_input_output_aliases=(),
            sim_require_finite=True,
            sim_require_nnan=True,
            nc=nc,
        )
        return tuple(outs)

    devices = jax.devices()[:N_CORES]
    mesh = Mesh(np.asarray(devices), ("core",))
    in_specs = (PartitionSpec("core"),) * (n_params + n_outs)
    out_specs = (PartitionSpec("core"),) * n_outs
    sharded = jax.jit(
        shard_map(_body, mesh=mesh, in_specs=in_specs, out_specs=out_specs,
                  check_rep=False),
        donate_argnums=donate, keep_unused=True)

    def run(in_maps):
        concat_in = [np.concatenate([np.asarray(m[name]) for m in in_maps], axis=0)
                     for name in in_names]
        concat_zeros = [np.zeros((N_CORES * s[0], *s[1:]), d) for s, d in zero_shapes]
        out_arrs = sharded(*concat_in, *concat_zeros)
        out_arrs = [np.asarray(a) for a in jax.block_until_ready(out_arrs)]
        return [
            {name: out_arrs[i].reshape(N_CORES, *out_avals[i].shape)[c]
             for i, name in enumerate(out_names)}
            for c in range(N_CORES)
        ]

    _CACHE[("internals", repeat, unroll)] = {
        "sharded": sharded, "mesh": mesh, "in_names": in_names,
        "out_names": out_names, "zero_shapes": zero_shapes, "nc": nc,
    }
    _CACHE[key] = run
    return run


def _prep_in_maps(query, key_value, Wq, bq, Wk, bk, Wv, bv, Wo, bo):
    f = np.float32
    bf = np.float16
    in_maps = []
    for c in range(N_CORES):
        b, hg = c // 2, c % 2
        sl = slice(hg * DC, (hg + 1) * DC)
        wvh = np.asarray(Wv, f)[sl, :].T
        bvh = np.asarray(bv, f)[sl]
        in_maps.append({
            "qT": np.ascontiguousarray(np.asarray(query, f)[b].T).astype(bf),
            "kvT": np.ascontiguousarray(np.asarray(key_value, f)[b].T).astype(bf),
            "wqT": np.ascontiguousarray(np.asarray(Wq, f)[sl, :].T).astype(bf),
            "wkT": np.ascontiguousarray(np.asarray(Wk, f)[sl, :].T).astype(bf),
            "wvh": np.ascontiguousarray(wvh).astype(bf),
            "bq": np.ascontiguousarray(np.asarray(bq, f)[sl]),
            "bk": np.ascontiguousarray(np.asarray(bk, f)[sl]),
            "bvh": np.ascontiguousarray(bvh).astype(bf),
            "woT": np.ascontiguousarray(np.asarray(Wo, f)[:, sl].T).astype(bf),
            "bo": (np.asarray(bo, f) if hg == 0 else np.zeros(D, f)).astype(bf),
        })
    return in_maps


def kernel(query, key_value, Wq, bq, Wk, bk, Wv, bv, Wo, bo):
    run = _get_runner()
    in_maps = _prep_in_maps(query, key_value, Wq, bq, Wk, bk, Wv, bv, Wo, bo)
    results = run(in_maps)
    out = np.empty((B, S, D), np.float32)
    for b in range(B):
        out[b] = (results[2 * b]["out"].astype(np.float32)
                  + results[2 * b + 1]["out"].astype(np.float32))
    return out
